# revision 9
# baseline (speedup 1.0000x reference)
"""Trainium2 Bass kernel for NonLocalBlock (self-attention over conv features).

Math per sample b (data-parallel over batch, 1 sample per NeuronCore):
  x:      [256, 4096]   (Cin x H*W, H=W=64)
  theta = (w_theta*s)   @ x                      [64, 4096]
  phi   = maxpool2x2((w_phi*s) @ x)              [64, 1024]
  g     = maxpool2x2((w_g*s)   @ x)              [256, 1024]
  scores= theta^T @ phi                          [4096, 1024]
  beta  = softmax_m(scores)
  att   = g @ beta^T                             [256, 4096]
  out   = gamma/sqrt2 * (w_o*s) @ att + 1/sqrt2 * (w_res*s) @ x   [512, 4096]

Device pipeline (per core):
  B: convs (fp32r matmuls, K=128 chunks, N=512 tiles)
  C: maxpools (tensor_tensor max on DVE/gpsimd)
  D: g_p transpose via identity matmuls (bf16)
  E: per 512-column window:
     E1 scores (fp32r, K=64) -> softmax stats -> E=exp bf16 (ACT, accum Z)
     E2 E^T via matmuls against diag(1/Z) bf16  -> ET_sb
     E3 att = g_pT^T @ ET (bf16)                -> att_sb f32r
     E4 out = wo^T.T @ att + wres^T.T @ x (fp32r, accumulated in one PSUM)
"""
import sys
import math
from contextlib import ExitStack

sys.path.insert(0, "/opt/trn_rl_repo")

import numpy as np

import concourse.bass as bass
import concourse.tile as tile
from concourse import bacc, mybir
from concourse.masks import make_identity
from concourse.bass_utils import run_bass_kernel_spmd

F32 = mybir.dt.float32
F32R = mybir.dt.float32r
BF16 = mybir.dt.bfloat16
AX = mybir.AxisListType
ALU = mybir.AluOpType
ACTF = mybir.ActivationFunctionType

N_CORES = 8
CIN = 256            # input channels
CTH = 64             # theta/phi channels
CG = 256             # g channels
COUT = 512           # output channels
HW = 4096            # 64*64
HWP = 1024           # pooled 32*32
NWIN = 8             # windows of 512 query positions
WIN = 512
NCH = 4              # 128-query chunks per window
MCH = 8              # 128-key chunks

_CACHE = {}
LAST_EXEC_NS = None
LAST_TRACE_DIR = None


def _build():
    nc = bacc.Bacc("TRN2", target_bir_lowering=False, debug=False,
                   num_devices=N_CORES)
    x_d = nc.dram_tensor("x", [CIN, HW], F32, kind="ExternalInput").ap()
    wth_d = nc.dram_tensor("wth", [CIN, CTH], F32, kind="ExternalInput").ap()
    wph_d = nc.dram_tensor("wph", [CIN, CTH], F32, kind="ExternalInput").ap()
    wg_d = nc.dram_tensor("wg", [CIN, CG], F32, kind="ExternalInput").ap()
    wo_d = nc.dram_tensor("wo", [CG, COUT], F32, kind="ExternalInput").ap()
    wres_d = nc.dram_tensor("wres", [CIN, COUT], F32, kind="ExternalInput").ap()
    out_d = nc.dram_tensor("out", [COUT, HW], F32, kind="ExternalOutput").ap()

    with tile.TileContext(nc) as tc, ExitStack() as ctx:
        persist = ctx.enter_context(tc.tile_pool(name="persist", bufs=1))

        # ---------------- weights + x loads ----------------
        x_sb = [persist.tile([128, HW], F32R, tag=f"x{k}", name=f"x_sb{k}") for k in range(2)]
        for j in range(NWIN):
            for k in range(2):
                nc.sync.dma_start(
                    out=x_sb[k][:, j * WIN:(j + 1) * WIN],
                    in_=x_d[k * 128:(k + 1) * 128, j * WIN:(j + 1) * WIN].bitcast(F32R),
                )
        wth_sb = [persist.tile([128, CTH], F32R, tag=f"wth{k}", name=f"wth_sb{k}") for k in range(2)]
        wph_sb = [persist.tile([128, CTH], F32R, tag=f"wph{k}", name=f"wph_sb{k}") for k in range(2)]
        wg_sb = [persist.tile([128, CG], F32R, tag=f"wg{k}", name=f"wg_sb{k}") for k in range(2)]
        wo_sb = [persist.tile([128, COUT], F32R, tag=f"wo{k}", name=f"wo_sb{k}") for k in range(2)]
        wres_sb = [persist.tile([128, COUT], F32R, tag=f"wres{k}", name=f"wres_sb{k}") for k in range(2)]
        for k in range(2):
            sl = slice(k * 128, (k + 1) * 128)
            nc.sync.dma_start(out=wth_sb[k][:], in_=wth_d[sl, :].bitcast(F32R))
            nc.sync.dma_start(out=wph_sb[k][:], in_=wph_d[sl, :].bitcast(F32R))
            nc.sync.dma_start(out=wg_sb[k][:], in_=wg_d[sl, :].bitcast(F32R))
            nc.sync.dma_start(out=wo_sb[k][:], in_=wo_d[sl, :].bitcast(F32R))
            nc.sync.dma_start(out=wres_sb[k][:], in_=wres_d[sl, :].bitcast(F32R))

        ident = persist.tile([128, 128], BF16, tag="ident")
        make_identity(nc, ident[:])

        theta_sb = persist.tile([128, HW], F32R, tag="theta")
        phi_p2 = persist.tile([128, HWP], F32R, tag="phi_p2", name="phi_p2")
        phi_sb = persist.tile([CTH, HW], F32, tag="phi_sb", name="phi_sb")
        g_sb = [persist.tile([128, HW], BF16, tag=f"gsb{c}", name=f"g_sb{c}") for c in range(2)]
        phi_tmp = persist.tile([CTH, HW // 2], F32, tag="phi_tmp")
        phi_p = persist.tile([CTH, HWP], F32R, tag="phi_p")
        g_tmp = persist.tile([128, HW // 2], BF16, tag="g_tmp", name="g_tmp")
        g_p = [persist.tile([128, HWP], BF16, tag=f"gp{c}", name=f"g_p{c}") for c in range(2)]
        # g_pT[mi]: [128 keys, 256 g-channels] bf16
        g_pT = [persist.tile([128, CG], BF16, tag=f"gpT{mi}", name=f"g_pT{mi}") for mi in range(MCH)]
        g_tmp2 = persist.tile([128, HW // 2], BF16, tag="g_tmp2", name="g_tmp2")

        # ---------------- stage B: convs (+ fused pool step 1 for phi/g) ----------------
        with tc.tile_pool(name="convps", bufs=2, space="PSUM") as convps, \
             tc.tile_pool(name="gps", bufs=4, space="PSUM") as gps:
            for j in range(NWIN):
                jsl = slice(j * WIN, (j + 1) * WIN)
                th_ps = convps.tile([CTH, WIN], F32, tag="th_ps")
                ph_ps = convps.tile([CTH, WIN], F32, tag="ph_ps")
                g_ps = [gps.tile([128, WIN], F32, tag="g_ps", name=f"g_ps{j}_{c}") for c in range(2)]
                for k in range(2):
                    st, sp = (k == 0), (k == 1)
                    nc.tensor.matmul(th_ps[:], wth_sb[k][:], x_sb[k][:, jsl],
                                     start=st, stop=sp)
                    nc.tensor.matmul(ph_ps[:], wph_sb[k][:], x_sb[k][:, jsl],
                                     start=st, stop=sp)
                    for c in range(2):
                        nc.tensor.matmul(
                            g_ps[c][:], wg_sb[k][:, c * 128:(c + 1) * 128],
                            x_sb[k][:, jsl], start=st, stop=sp)
                nc.scalar.copy(theta_sb[0:CTH, jsl], th_ps[:])
                nc.sync.dma_start(out=theta_sb[CTH:128, jsl],
                                  in_=theta_sb[0:CTH, jsl])
                nc.scalar.copy(phi_sb[:, jsl], ph_ps[:])
                for c in range(2):
                    nc.vector.tensor_copy(g_sb[c][:, jsl], g_ps[c][:])
                # pool step 1 (pairs along w) from SBUF
                hsl = slice(j * (WIN // 2), (j + 1) * (WIN // 2))
                php = phi_sb[:, jsl].rearrange("p (a two) -> p a two", two=2)
                nc.vector.tensor_tensor(out=phi_tmp[:, hsl], in0=php[:, :, 0],
                                        in1=php[:, :, 1], op=ALU.max)
                for c in range(2):
                    gpp = g_sb[c][:, jsl].rearrange("p (a two) -> p a two", two=2)
                    dst = g_tmp if c == 0 else g_tmp2
                    nc.vector.tensor_tensor(out=dst[:, hsl], in0=gpp[:, :, 0],
                                            in1=gpp[:, :, 1], op=ALU.max)

        # ---------------- stage C: pool step 2 (pairs along h) ----------------
        ph_rows = phi_tmp[:].rearrange("p (h2 two w2) -> p h2 two w2",
                                       h2=32, two=2, w2=32)
        nc.vector.tensor_tensor(out=phi_p[:], in0=ph_rows[:, :, 0, :],
                                in1=ph_rows[:, :, 1, :], op=ALU.max)
        nc.sync.dma_start(out=phi_p2[0:CTH, :], in_=phi_p[:])
        nc.sync.dma_start(out=phi_p2[CTH:128, :], in_=phi_p[:])
        for c in range(2):
            srct = g_tmp if c == 0 else g_tmp2
            gp_rows = srct[:].rearrange("p (h2 two w2) -> p h2 two w2",
                                        h2=32, two=2, w2=32)
            nc.vector.tensor_tensor(out=g_p[c][:], in0=gp_rows[:, :, 0, :],
                                    in1=gp_rows[:, :, 1, :], op=ALU.max)

        # ---------------- stage D: g_p transpose ----------------
        with tc.tile_pool(name="gtps", bufs=2, space="PSUM") as gtps:
            for mi in range(MCH):
                msl = slice(mi * 128, (mi + 1) * 128)
                gt_ps = gtps.tile([128, CG], F32, tag="gt_ps")
                for c in range(2):
                    nc.tensor.matmul(gt_ps[:, c * 128:(c + 1) * 128],
                                     g_p[c][:, msl], ident[:],
                                     start=True, stop=True)
                nc.scalar.copy(g_pT[mi][:], gt_ps[:])

        # ---------------- stage E: attention + output, software-pipelined ----------------
        scps = ctx.enter_context(tc.tile_pool(name="scps", bufs=2, space="PSUM"))
        etps = ctx.enter_context(tc.tile_pool(name="etps", bufs=2, space="PSUM"))
        attps = ctx.enter_context(tc.tile_pool(name="attps", bufs=1, space="PSUM"))
        finps = ctx.enter_context(tc.tile_pool(name="finps", bufs=1, space="PSUM"))
        epool = ctx.enter_context(tc.tile_pool(name="epool", bufs=8))
        dpool = ctx.enter_context(tc.tile_pool(name="dpool", bufs=8))
        etsb = ctx.enter_context(tc.tile_pool(name="etsb", bufs=2 * MCH))
        attsb = ctx.enter_context(tc.tile_pool(name="attsb", bufs=4))
        finsb = ctx.enter_context(tc.tile_pool(name="finsb", bufs=4))
        stats = ctx.enter_context(tc.tile_pool(name="stats", bufs=2 * NCH))

        def emit_E1_pair(w, pair):
            """scores + softmax stats + E + diag for chunks (2*pair, 2*pair+1).
            The two chunks share the PE array via row groups (0,0)/(64,0)."""
            sps = []
            for jc in (2 * pair, 2 * pair + 1):
                n0 = w * WIN + jc * 128
                base = 0 if jc % 2 == 0 else CTH
                s_ps = scps.tile([128, HWP], F32, tag="s_ps",
                                 name=f"s_ps{w}_{jc}")
                for mt in range(2):
                    nc.tensor.matmul(
                        s_ps[:, mt * 512:(mt + 1) * 512],
                        theta_sb[base:base + CTH, n0:n0 + 128],
                        phi_p2[base:base + CTH, mt * 512:(mt + 1) * 512],
                        start=True, stop=True)
                sps.append(s_ps)
            chunks = []
            for i, jc in enumerate((2 * pair, 2 * pair + 1)):
                s_ps = sps[i]
                negmx = stats.tile([128, 1], F32, tag="negmx",
                                   name=f"negmx{w}_{jc}")
                nc.vector.reduce_max(negmx[:], s_ps[:], axis=AX.X, negate=True)
                E_sb = epool.tile([128, HWP], BF16, tag="E", name=f"E{w}_{jc}")
                z = stats.tile([128, 1], F32, tag="z", name=f"z{w}_{jc}")
                nc.scalar.activation(E_sb[:], s_ps[:], ACTF.Exp,
                                     bias=negmx[:], scale=1.0, accum_out=z[:])
                rz = stats.tile([128, 1], F32, tag="rz", name=f"rz{w}_{jc}")
                nc.vector.reciprocal(rz[:], z[:])
                diag = dpool.tile([128, 128], BF16, tag="diag",
                                  name=f"diag{w}_{jc}")
                nc.vector.tensor_scalar_mul(diag[:], ident[:], rz[:])
                chunks.append((E_sb, diag))
            return chunks

        def emit_E2_half(w, chunks, half):
            ET = []
            for mi in range(half * (MCH // 2), (half + 1) * (MCH // 2)):
                msl = slice(mi * 128, (mi + 1) * 128)
                et_ps = etps.tile([128, WIN], F32, tag="et_ps",
                                  name=f"et_ps{w}_{mi}")
                for jc in range(NCH):
                    nc.tensor.matmul(
                        et_ps[:, jc * 128:(jc + 1) * 128],
                        chunks[jc][0][:, msl], chunks[jc][1][:],
                        start=True, stop=True)
                et_sb = etsb.tile([128, WIN], BF16, tag="et_sb",
                                  name=f"et_sb{w}_{mi}")
                nc.vector.tensor_copy(et_sb[:], et_ps[:])
                ET.append(et_sb)
            return ET

        def emit_E34_mms(w, ET):
            wsl = slice(w * WIN, (w + 1) * WIN)
            att_chunks = []
            for c in range(2):
                att_ps = attps.tile([128, WIN], F32, tag="att_ps",
                                    name=f"att_ps{w}_{c}")
                for mi in range(MCH):
                    nc.tensor.matmul(
                        att_ps[:], g_pT[mi][:, c * 128:(c + 1) * 128],
                        ET[mi][:],
                        start=(mi == 0), stop=(mi == MCH - 1))
                att_sb = attsb.tile([128, WIN], F32R, tag="att_sb",
                                    name=f"att_sb{w}_{c}")
                nc.scalar.copy(att_sb[:], att_ps[:])
                att_chunks.append(att_sb)
            fps = []
            for oc in range(4):
                osl = slice(oc * 128, (oc + 1) * 128)
                f_ps = finps.tile([128, WIN], F32, tag="f_ps",
                                  name=f"f_ps{w}_{oc}")
                nc.tensor.matmul(f_ps[:], wo_sb[0][:, osl], att_chunks[0][:],
                                 start=True, stop=False)
                nc.tensor.matmul(f_ps[:], wo_sb[1][:, osl], att_chunks[1][:],
                                 start=False, stop=False)
                nc.tensor.matmul(f_ps[:], wres_sb[0][:, osl], x_sb[0][:, wsl],
                                 start=False, stop=False)
                nc.tensor.matmul(f_ps[:], wres_sb[1][:, osl], x_sb[1][:, wsl],
                                 start=False, stop=True)
                fps.append(f_ps)
            return fps

        def emit_fin_evicts(w, fps):
            wsl = slice(w * WIN, (w + 1) * WIN)
            for oc in range(4):
                osl = slice(oc * 128, (oc + 1) * 128)
                f_sb = finsb.tile([128, WIN], F32, tag="f_sb",
                                  name=f"f_sb{w}_{oc}")
                nc.scalar.copy(f_sb[:], fps[oc][:])
                nc.sync.dma_start(out=out_d[osl, wsl], in_=f_sb[:])

        # software pipeline: window w+1's scores/softmax emitted in pair-
        # slices inside window w's transpose stream; window w's final-output
        # evictions are deferred to iteration w+1 so they hit ready PSUMs and
        # never block the exp ops queued behind them on the scalar engine.
        chunks = emit_E1_pair(0, 0) + emit_E1_pair(0, 1)
        pend = None
        for w in range(NWIN):
            if pend is not None:
                emit_fin_evicts(*pend)
            nxt = []
            if w + 1 < NWIN:
                nxt += emit_E1_pair(w + 1, 0)
            ET = emit_E2_half(w, chunks, 0)
            if w + 1 < NWIN:
                nxt += emit_E1_pair(w + 1, 1)
            ET += emit_E2_half(w, chunks, 1)
            pend = (w, emit_E34_mms(w, ET))
            chunks = nxt
        emit_fin_evicts(*pend)

    nc.compile()
    return nc


def kernel(**inputs):
    global LAST_EXEC_NS
    x = np.asarray(inputs["x"], dtype=np.float32)          # [8, 256, 64, 64]
    w_theta = np.asarray(inputs["w_theta"], np.float32)    # [64, 256]
    w_phi = np.asarray(inputs["w_phi"], np.float32)
    w_g = np.asarray(inputs["w_g"], np.float32)            # [256, 256]
    w_o = np.asarray(inputs["w_o"], np.float32)            # [512, 256]
    w_res = np.asarray(inputs["w_res"], np.float32)        # [512, 256]
    gamma = float(np.asarray(inputs["gamma"]).reshape(-1)[0])

    s = math.sqrt(2.0 / 256.0)
    inv_sqrt2 = 1.0 / math.sqrt(2.0)
    wth = np.ascontiguousarray((w_theta * s).T)            # [256, 64]
    wph = np.ascontiguousarray((w_phi * s).T)
    wg = np.ascontiguousarray((w_g * s).T)                 # [256, 256]
    wo = np.ascontiguousarray((w_o * (s * gamma * inv_sqrt2)).T)   # [256, 512]
    wres = np.ascontiguousarray((w_res * (s * inv_sqrt2)).T)       # [256, 512]

    if "nc" not in _CACHE:
        _CACHE["nc"] = _build()
    nc = _CACHE["nc"]

    B = x.shape[0]
    xb = x.reshape(B, CIN, HW)
    in_maps = [{
        "x": np.ascontiguousarray(xb[b]),
        "wth": wth, "wph": wph, "wg": wg, "wo": wo, "wres": wres,
    } for b in range(B)]

    r = run_bass_kernel_spmd(nc, in_maps, core_ids=list(range(N_CORES)))
    if r.exec_time_ns is not None:
        LAST_EXEC_NS = r.exec_time_ns
    out = np.stack([r.results[b]["out"] for b in range(B)])
    return out.reshape(B, COUT, 64, 64).astype(np.float32)


def _prep_in_maps(inputs):
    x = np.asarray(inputs["x"], dtype=np.float32)
    w_theta = np.asarray(inputs["w_theta"], np.float32)
    w_phi = np.asarray(inputs["w_phi"], np.float32)
    w_g = np.asarray(inputs["w_g"], np.float32)
    w_o = np.asarray(inputs["w_o"], np.float32)
    w_res = np.asarray(inputs["w_res"], np.float32)
    gamma = float(np.asarray(inputs["gamma"]).reshape(-1)[0])
    s = math.sqrt(2.0 / 256.0)
    inv_sqrt2 = 1.0 / math.sqrt(2.0)
    wth = np.ascontiguousarray((w_theta * s).T)
    wph = np.ascontiguousarray((w_phi * s).T)
    wg = np.ascontiguousarray((w_g * s).T)
    wo = np.ascontiguousarray((w_o * (s * gamma * inv_sqrt2)).T)
    wres = np.ascontiguousarray((w_res * (s * inv_sqrt2)).T)
    B = x.shape[0]
    xb = x.reshape(B, CIN, HW)
    return [{
        "x": np.ascontiguousarray(xb[b]),
        "wth": wth, "wph": wph, "wg": wg, "wo": wo, "wres": wres,
    } for b in range(B)]


def kernel_profiled(**inputs):
    """Run with NTFF tracing; sets LAST_EXEC_NS / LAST_TRACE_DIR."""
    global LAST_EXEC_NS, LAST_TRACE_DIR
    import tempfile
    if "nc" not in _CACHE:
        _CACHE["nc"] = _build()
    nc = _CACHE["nc"]
    in_maps = _prep_in_maps(inputs)
    tmpdir = tempfile.mkdtemp(prefix="nlb_trace_")
    r = run_bass_kernel_spmd(nc, in_maps, core_ids=list(range(N_CORES)),
                             trace=True, tmpdir=tmpdir)
    LAST_TRACE_DIR = tmpdir
    if r.exec_time_ns is not None:
        LAST_EXEC_NS = r.exec_time_ns
    B = len(in_maps)
    out = np.stack([r.results[b]["out"] for b in range(B)])
    return out.reshape(B, COUT, 64, 64).astype(np.float32)
